# revision 2
# baseline (speedup 1.0000x reference)
"""ContextualLoss forward on 8 Trainium2 NeuronCores — v2.

Math (reference):
    mu[m]   = mean_c Y[c, m]                      (PONO over channels of Y)
    Xc = X - mu ; Yc = Y - mu                     (both centered by Y's mean)
    cos[i,j] = <Xc_i, Yc_j> / (|Xc_i| |Yc_j|)
    d = 1 - cos ; dn = d / (min_j d + 1e-3) ; w = exp((1 - dn)/0.1)
    A = w / sum_j w ; CX_b = mean_i max_j A ; loss = mean_b -log CX_b

v2 structure (per core: one sample b, one 2048-row half):
  * Inputs are host-cast to bf16: input DMA bytes halve and the on-device
    f32->bf16 conversion passes disappear.
  * Only Y is centered (<Xc_i, Yc_j> == <X_i, Yc_j>), then scaled by
    1/|Yc_j| in prep, so the main-loop matmul yields s_ij = cos * |Xc_i|
    and every PSUM eviction is a pure copy.
  * Fused evict+rowmax: the custom-DVE TENSOR_MASK_REDUCE op (uOps table)
    copies a PSUM quarter to bf16 SBUF AND computes the f32 row max in one
    1192ns instruction; accum chains across quarters via s1. (The native
    TENSOR_TENSOR_REDUCE / TENSOR_MASK_REDUCE ISA ops hard-fault this
    runtime; the custom-DVE version verified exact on HW.)
  * Per tile: DVE runs 3 TMRs; ScalarE evicts quarter 3 (Identity) and does
    the fused Exp (exponent = (10 r/|Xc_i|) * s + 10 - 10r via per-partition
    scale/bias APs, accum_out = sum_j w); DVE does a small bf16 2x pairwise
    tree for quarter 3's max. GPSIMD (SBUF-only: no PSUM, no max) takes the
    per-tile scale_i multiply.
  * max_j A = exp(0.01 r) / sum_j w  (w monotone in d).
  * |Yc| scaling uses Abs_reciprocal_sqrt straight from PSUM (verified
    ~2e-3 on HW; well within tolerance), skipping the DVE reciprocal.
  * |Xc_i|^2 = sum X^2 - (sy/256)*(2*sum X - sy) via tiny N=1 column-sum
    matmuls -- Xc is never materialized.
  * Keep-alive matmuls park in a sacrificial PSUM bank so the PE p-state
    never down-clocks between bursts.

Sharding: core c -> sample b = c//2, row-half h = c%2 (2048 rows each).
Each core's Y is column-permuted host-side to [own-half | other-half] so the
identical SPMD program reads the X-half's stats from columns [0, 2048).
Row reductions are permutation-invariant, so the permutation is harmless.
"""

import sys

sys.path.insert(0, "/opt/trn_rl_repo")

import numpy as np
import ml_dtypes

import concourse.bass as bass
import concourse.tile as tile
from concourse import bacc
from concourse import mybir
from concourse.bass_utils import run_bass_kernel_spmd
from concourse.dve_ops import TENSOR_MASK_REDUCE

B = 4
C = 256
M = 4096  # 64*64 spatial positions
HALF = M // 2  # rows per core
NT = HALF // 128  # 16 i-tiles per core
N_CORES = 8
Q = 1024  # quarter width

F32 = mybir.dt.float32
BF16 = mybir.dt.bfloat16
AF = mybir.ActivationFunctionType
ALU = mybir.AluOpType


def build_nc() -> bass.Bass:
    nc = bacc.Bacc()

    x_d = nc.declare_dram_parameter("x", [C, HALF], BF16, isOutput=False)
    y_d = nc.declare_dram_parameter("y", [C, M], BF16, isOutput=False)
    v_d = nc.declare_dram_parameter("v", [128, NT], F32, isOutput=True)

    with tile.TileContext(nc) as tc:
        with (
            tc.tile_pool(name="io", bufs=1) as io,
            tc.tile_pool(name="consts", bufs=1) as consts,
            tc.tile_pool(name="stats", bufs=1) as stats,
            tc.tile_pool(name="psum_ka", bufs=1, space="PSUM") as psum_ka,
        ):
            y_q = [
                io.tile([128, 2, Q], BF16, name=f"y_q{i}") for i in range(4)
            ]
            x_sb = io.tile([128, 2, HALF], BF16)

            ones_col_bf = consts.tile([128, 1], BF16)
            nc.vector.memset(ones_col_bf, 1.0)
            bc_inv256 = consts.tile([128, 128], BF16)  # rank-reduce+bcast mu
            nc.vector.memset(bc_inv256, 1.0 / 256.0)
            bc_ones = consts.tile([128, 128], BF16)  # rank-reduce+bcast qy
            nc.vector.memset(bc_ones, 1.0)
            c1001_col = consts.tile([128, 1], F32)
            nc.vector.memset(c1001_col, 1.001)
            me_col = consts.tile([128, 1], F32)  # TMR mask_end: cover all
            nc.vector.memset(me_col, 1e9)
            ones_512 = consts.tile([128, 512], BF16)
            nc.vector.memset(ones_512, 1.0)

            sy16 = stats.tile([128, NT], F32)  # sum_c Y over own-half cols
            sx16 = stats.tile([128, NT], F32)  # sum_c X
            sxx16 = stats.tile([128, NT], F32)  # sum_c X^2
            nx2 = stats.tile([128, NT], F32)
            inv_nx = stats.tile([128, NT], F32)
            neg_inv_nx = stats.tile([128, NT], F32)
            ten_inv_nx = stats.tile([128, NT], F32)
            r16 = stats.tile([128, NT], F32)
            sumwA = stats.tile([128, NT], F32)
            maxw16 = stats.tile([128, NT], F32)
            rs16 = stats.tile([128, NT], F32)
            v16 = stats.tile([128, NT], F32)
            t16 = stats.tile([128, NT], F32)

            y_v = y_d.rearrange("(k p) m -> p k m", p=128)
            x_v = x_d.rearrange("(k p) m -> p k m", p=128)

            # keep-alive PSUM bank: dependency-free filler matmuls keep the
            # PE p-state from down-clocking between real bursts
            ka_ps = psum_ka.tile([128, 512], F32)

            def ka(n):
                for _ in range(n):
                    nc.tensor.matmul(
                        ka_ps[:, :], lhsT=bc_ones[:, :], rhs=ones_512[:, :],
                        start=True, stop=True,
                    )

            def tmr(out_ap, in_ap, accum_in, accum_out):
                nc.vector._custom_dve(
                    TENSOR_MASK_REDUCE,
                    out=out_ap, in0=in_ap, in1=me_col[:, :],
                    s0=0.0, s1=accum_in, imm2=1.0, accum_out=accum_out,
                )

            with (
                tc.tile_pool(name="pre", bufs=3, space="PSUM") as pre,
                tc.tile_pool(name="pstat", bufs=1, space="PSUM") as pstat,
                tc.tile_pool(name="scr", bufs=3) as scr,
            ):
                # ---- input DMAs: quarters land in arrival order; spread
                # descriptor-gen across HWDGE queues (transfers serialize on
                # the DMA engines anyway)
                for q in range(4):
                    eng = nc.sync if q % 2 == 0 else nc.scalar
                    eng.dma_start(
                        out=y_q[q][:, :, :], in_=y_v[:, :, q * Q : (q + 1) * Q]
                    )
                nc.gpsimd.dma_start(out=x_sb[:, :, 0:Q], in_=x_v[:, :, 0:Q])
                nc.gpsimd.dma_start(out=x_sb[:, :, Q:HALF], in_=x_v[:, :, Q:HALF])

                def stat16(dst, src_sb, tiles, ones):
                    """dst[p, t] = sum_c src[c, (t-tiles[0])*128 + p]."""
                    ps = pstat.tile([128, len(tiles)], F32, tag="pstat")
                    for i, t in enumerate(tiles):
                        for k in range(2):
                            nc.tensor.matmul(
                                ps[:, i : i + 1],
                                lhsT=src_sb[:, k, i * 128 : (i + 1) * 128],
                                rhs=ones[:, :],
                                start=(k == 0),
                                stop=(k == 1),
                            )
                    nc.scalar.activation(
                        dst[:, tiles[0] : tiles[0] + len(tiles)], ps[:, :],
                        AF.Copy,
                    )

                def center_mm(q):
                    # mu[p, j] = sum_c y[c, j] / 256 for every partition p
                    ps = pre.tile([128, Q], F32, tag="pre")
                    for j in range(2):
                        for k in range(2):
                            nc.tensor.matmul(
                                ps[:, j * 512 : (j + 1) * 512],
                                lhsT=bc_inv256[:, :],
                                rhs=y_q[q][:, k, j * 512 : (j + 1) * 512],
                                start=(k == 0),
                                stop=(k == 1),
                            )
                    return ps

                def center_sub_dve(q, ps):
                    for k in range(2):
                        nc.vector.tensor_sub(
                            y_q[q][:, k, :], y_q[q][:, k, :], ps[:, :]
                        )

                def center_sub_pool(q, ps):
                    # GPSIMD can't read PSUM: ScalarE stages mu to SBUF bf16
                    mu_bf = scr.tile([128, Q], BF16, tag="mubf")
                    nc.scalar.activation(mu_bf[:, :], ps[:, :], AF.Copy)
                    for k in range(2):
                        nc.gpsimd.tensor_sub(
                            y_q[q][:, k, :], y_q[q][:, k, :], mu_bf[:, :]
                        )

                def sq_quarter(q, on_dve):
                    sq = scr.tile([128, 2, Q], BF16, tag="sq")
                    src = y_q[q][:, :, :]
                    if on_dve:
                        nc.vector.tensor_mul(sq[:, :, :], src, src)
                    else:
                        nc.scalar.activation(sq[:, :, :], src, AF.Square)
                    return sq

                def ny_quarter(q, sq, scale_pool=False):
                    # qy[p, j] = sum_c Yc[c, j]^2 broadcast via ones lhsT;
                    # inv_ny = 1/sqrt(qy) in one ScalarE op from PSUM, then
                    # y_q *= inv_ny (bf16 2x)
                    ps = pre.tile([128, Q], F32, tag="pre")
                    for j in range(2):
                        for k in range(2):
                            nc.tensor.matmul(
                                ps[:, j * 512 : (j + 1) * 512],
                                lhsT=bc_ones[:, :],
                                rhs=sq[:, k, j * 512 : (j + 1) * 512],
                                start=(k == 0),
                                stop=(k == 1),
                            )
                    ny_bf = scr.tile([128, Q], BF16, tag="nybf")
                    nc.scalar.activation(
                        ny_bf[:, :], ps[:, :], AF.Abs_reciprocal_sqrt
                    )
                    eng = nc.gpsimd if scale_pool else nc.vector
                    for k in range(2):
                        eng.tensor_mul(
                            y_q[q][:, k, :], y_q[q][:, k, :], ny_bf[:, :]
                        )

                # ---- phase schedule -----------------------------------
                # dummy activations: pull the Exp/Abs_reciprocal_sqrt act
                # tables in during the DMA wait instead of mid-pipeline
                dum = scr.tile([128, 1], F32, tag="dum")
                nc.scalar.activation(
                    dum[:, :], c1001_col[:, :], AF.Abs_reciprocal_sqrt
                )
                # own-half raw-Y column sums (must precede centering)
                cps = [None] * 4
                cps[0] = center_mm(0)
                stat16(sy16, y_q[0], list(range(0, 8)), ones_col_bf)
                center_sub_dve(0, cps[0])
                ka(10)  # ramp the PE clock behind the first mu burst
                cps[1] = center_mm(1)
                stat16(sy16, y_q[1], list(range(8, 16)), ones_col_bf)
                center_sub_dve(1, cps[1])
                cps[2] = center_mm(2)
                center_sub_pool(2, cps[2])
                cps[3] = center_mm(3)
                center_sub_pool(3, cps[3])
                sq0 = sq_quarter(0, on_dve=True)
                sq1 = sq_quarter(1, on_dve=False)
                sq2 = sq_quarter(2, on_dve=False)
                sq3 = sq_quarter(3, on_dve=True)
                ny_quarter(0, sq0)
                ny_quarter(1, sq1)
                ny_quarter(2, sq2)
                ny_quarter(3, sq3)
                # X stats: x lands last; deprioritize so early main-loop
                # work can slot ahead in the engine queues
                with tc.tile_wait_until(0.0105):
                    sqx = scr.tile([128, 2, HALF], BF16, tag="sqx")
                    nc.vector.tensor_mul(sqx[:, :, :], x_sb[:, :, :], x_sb[:, :, :])
                    stat16(sx16, x_sb, list(range(NT)), ones_col_bf)
                    stat16(sxx16, sqx, list(range(NT)), ones_col_bf)
                # nx2 = sxx - (sy/256)*(2*sx - sy)  (tiny DVE ops)
                nc.vector.tensor_scalar(
                    out=t16[:, :], in0=sx16[:, :], scalar1=2.0, scalar2=None,
                    op0=ALU.mult,
                )
                nc.vector.tensor_sub(t16[:, :], t16[:, :], sy16[:, :])
                nc.vector.tensor_mul(t16[:, :], t16[:, :], sy16[:, :])
                nc.vector.tensor_scalar(
                    out=t16[:, :], in0=t16[:, :], scalar1=1.0 / 256.0,
                    scalar2=None, op0=ALU.mult,
                )
                nc.vector.tensor_sub(nx2[:, :], sxx16[:, :], t16[:, :])
                nc.scalar.activation(
                    inv_nx[:, :], nx2[:, :], AF.Abs_reciprocal_sqrt
                )  # 1/|Xc|
                nc.vector.tensor_scalar(
                    out=neg_inv_nx[:, :], in0=inv_nx[:, :], scalar1=-1.0,
                    scalar2=None, op0=ALU.mult,
                )
                nc.vector.tensor_scalar(
                    out=ten_inv_nx[:, :], in0=inv_nx[:, :], scalar1=10.0,
                    scalar2=None, op0=ALU.mult,
                )

            # ---- main loop -------------------------------------------------
            with (
                tc.tile_pool(name="psum_g", bufs=3, space="PSUM") as psum_g,
                tc.tile_pool(name="dpool", bufs=4) as dpool,
                tc.tile_pool(name="wpool", bufs=1) as wpool,
                tc.tile_pool(name="mpool", bufs=4) as mpool,
                tc.tile_pool(name="mains", bufs=10) as mains,
            ):

                def quarter_mm(t, g):
                    ps = psum_g.tile([128, Q], F32, tag="g")
                    for k in range(2):
                        for j in range(2):
                            nc.tensor.matmul(
                                ps[:, j * 512 : (j + 1) * 512],
                                lhsT=x_sb[:, k, t * 128 : (t + 1) * 128],
                                rhs=y_q[g][:, k, j * 512 : (j + 1) * 512],
                                start=(k == 0),
                                stop=(k == 1),
                            )
                    return ps

                def q3_tree(t, d_sb):
                    # quarter 3 (Act-evicted, no fused max): pairwise bf16
                    # 2x tree + short reduce
                    m1 = mpool.tile([128, 512], BF16, tag="m1")
                    m2 = mpool.tile([128, 128], BF16, tag="m2")
                    mx3 = mains.tile([128, 1], F32, tag="mx3")
                    nc.vector.tensor_tensor(
                        out=m1[:, :], in0=d_sb[:, 3072:3584],
                        in1=d_sb[:, 3584:4096], op=ALU.max,
                    )
                    nc.vector.tensor_tensor(
                        out=m2[:, :], in0=m1[:, 0:128], in1=m1[:, 128:256],
                        op=ALU.max,
                    )
                    nc.vector.tensor_tensor(
                        out=m2[:, :], in0=m2[:, :], in1=m1[:, 256:384],
                        op=ALU.max,
                    )
                    nc.vector.tensor_tensor(
                        out=m2[:, :], in0=m2[:, :], in1=m1[:, 384:512],
                        op=ALU.max,
                    )
                    nc.vector.reduce_max(
                        mx3, m2[:, :], axis=mybir.AxisListType.X
                    )
                    return mx3

                def exp_tile(t, d_sb, scale_i, bias_i):
                    w_sb = wpool.tile([128, M], BF16, tag="w")
                    nc.scalar.activation(
                        out=w_sb[:, :],
                        in_=d_sb[:, :],
                        func=AF.Exp,
                        bias=bias_i,
                        scale=scale_i,
                        accum_out=sumwA[:, t : t + 1],
                    )

                prev = None
                ps_pend = [quarter_mm(0, g) for g in range(3)]
                for t in range(NT):
                    d_sb = dpool.tile([128, M], BF16, tag="d")
                    ps0, ps1, ps2 = ps_pend
                    ka(2)
                    ps3 = quarter_mm(t, 3)
                    ka(2)
                    m0 = mains.tile([128, 1], F32, tag="m0")
                    m1c = mains.tile([128, 1], F32, tag="m1c")
                    m2c = mains.tile([128, 1], F32, tag="m2c")
                    tmr(d_sb[:, 0:Q], ps0[:, :], -1e30, m0)
                    tmr(d_sb[:, Q : 2 * Q], ps1[:, :], m0, m1c)
                    if prev is not None:
                        pt, pd, psc, pbi = prev
                        exp_tile(pt, pd, psc, pbi)
                    # ScalarE evicts quarter 3 (pure copy; exp runs first in
                    # the Act queue)
                    nc.scalar.activation(
                        d_sb[:, 3 * Q : M], ps3[:, :], AF.Copy
                    )
                    tmr(d_sb[:, 2 * Q : 3 * Q], ps2[:, :], m1c, m2c)
                    if t + 1 < NT:
                        ps_pend = [quarter_mm(t + 1, g) for g in range(3)]
                    mx3 = q3_tree(t, d_sb)
                    smax = mains.tile([128, 1], F32, tag="smax")
                    nc.vector.tensor_tensor(
                        out=smax, in0=m2c, in1=mx3, op=ALU.max
                    )
                    # u = 1.001 - smax/|Xc_i| ; r = 1/u
                    u = mains.tile([128, 1], F32, tag="u")
                    nc.vector.scalar_tensor_tensor(
                        out=u, in0=smax, scalar=neg_inv_nx[:, t : t + 1],
                        in1=c1001_col[:, :], op0=ALU.mult, op1=ALU.add,
                    )
                    nc.vector.reciprocal(r16[:, t : t + 1], u)
                    # scale_i = 10*r/|Xc_i| on GPSIMD; bias_i = 10 - 10r
                    scale_i = mains.tile([128, 1], F32, tag="scale")
                    bias_i = mains.tile([128, 1], F32, tag="bias")
                    nc.gpsimd.tensor_mul(
                        scale_i, r16[:, t : t + 1], ten_inv_nx[:, t : t + 1]
                    )
                    nc.vector.tensor_scalar(
                        out=bias_i, in0=r16[:, t : t + 1], scalar1=-10.0,
                        scalar2=10.0, op0=ALU.mult, op1=ALU.add,
                    )
                    prev = (t, d_sb, scale_i, bias_i)

                pt, pd, psc, pbi = prev
                exp_tile(pt, pd, psc, pbi)

                # ---- epilogue: v = exp(0.01*r) / sumw ---------------------
                nc.scalar.activation(maxw16[:, :], r16[:, :], AF.Exp, scale=0.01)
                nc.vector.reciprocal(rs16[:, :], sumwA[:, :])
                nc.vector.tensor_mul(v16[:, :], maxw16[:, :], rs16[:, :])
                nc.sync.dma_start(out=v_d[:, :], in_=v16[:, :])

    nc.compile()
    return nc


_NC = None


def _get_nc():
    global _NC
    if _NC is None:
        _NC = build_nc()
    return _NC


def make_in_maps(X, Y):
    """Per-core bf16 inputs. Y columns permuted to [own-half | other-half]."""
    Xb = X.astype(ml_dtypes.bfloat16)
    Yb = Y.astype(ml_dtypes.bfloat16)
    in_maps = []
    for c in range(N_CORES):
        b, h = c // 2, c % 2
        xs = np.ascontiguousarray(Xb[b][:, h * HALF : (h + 1) * HALF])
        ys = np.ascontiguousarray(
            np.concatenate(
                [
                    Yb[b][:, h * HALF : (h + 1) * HALF],
                    Yb[b][:, (1 - h) * HALF : (2 - h) * HALF],
                ],
                axis=1,
            )
        )
        in_maps.append({"x": xs, "y": ys})
    return in_maps


def finish_host(results):
    """results: list of 8 per-core dicts with 'v' [128, NT]."""
    cx = np.zeros(B, dtype=np.float64)
    for c in range(N_CORES):
        cx[c // 2] += np.asarray(results[c]["v"]).astype(np.float64).sum()
    cx /= M
    return np.float32(np.mean(-np.log(cx)))


def run(X_features, Y_features, trace=False, tmpdir=None):
    X = np.asarray(X_features, dtype=np.float32).reshape(B, C, M)
    Y = np.asarray(Y_features, dtype=np.float32).reshape(B, C, M)
    nc = _get_nc()
    res = run_bass_kernel_spmd(
        nc, make_in_maps(X, Y), list(range(N_CORES)), trace=trace, tmpdir=tmpdir
    )
    return finish_host(res.results), res


def kernel(X_features, Y_features):
    loss, _ = run(X_features, Y_features)
    return loss


# revision 3
# speedup vs baseline: 1.0094x; 1.0094x over previous
"""ContextualLoss forward on 8 Trainium2 NeuronCores — v2.

Math (reference):
    mu[m]   = mean_c Y[c, m]                      (PONO over channels of Y)
    Xc = X - mu ; Yc = Y - mu                     (both centered by Y's mean)
    cos[i,j] = <Xc_i, Yc_j> / (|Xc_i| |Yc_j|)
    d = 1 - cos ; dn = d / (min_j d + 1e-3) ; w = exp((1 - dn)/0.1)
    A = w / sum_j w ; CX_b = mean_i max_j A ; loss = mean_b -log CX_b

v2 structure (per core: one sample b, one 2048-row half):
  * Inputs are host-cast to bf16: input DMA bytes halve and the on-device
    f32->bf16 conversion passes disappear.
  * Only Y is centered (<Xc_i, Yc_j> == <X_i, Yc_j>), then scaled by
    1/|Yc_j| in prep, so the main-loop matmul yields s_ij = cos * |Xc_i|
    and every PSUM eviction is a pure copy.
  * Fused evict+rowmax: the custom-DVE TENSOR_MASK_REDUCE op (uOps table)
    copies a PSUM quarter to bf16 SBUF AND computes the f32 row max in one
    1192ns instruction; accum chains across quarters via s1. (The native
    TENSOR_TENSOR_REDUCE / TENSOR_MASK_REDUCE ISA ops hard-fault this
    runtime; the custom-DVE version verified exact on HW.)
  * Per tile: DVE runs 3 TMRs; ScalarE evicts quarter 3 (Identity) and does
    the fused Exp (exponent = (10 r/|Xc_i|) * s + 10 - 10r via per-partition
    scale/bias APs, accum_out = sum_j w); DVE does a small bf16 2x pairwise
    tree for quarter 3's max. GPSIMD (SBUF-only: no PSUM, no max) takes the
    per-tile scale_i multiply.
  * max_j A = exp(0.01 r) / sum_j w  (w monotone in d).
  * |Yc| scaling uses Abs_reciprocal_sqrt straight from PSUM (verified
    ~2e-3 on HW; well within tolerance), skipping the DVE reciprocal.
  * |Xc_i|^2 = sum X^2 - (sy/256)*(2*sum X - sy) via tiny N=1 column-sum
    matmuls -- Xc is never materialized.
  * Keep-alive matmuls park in a sacrificial PSUM bank so the PE p-state
    never down-clocks between bursts.

Sharding: core c -> sample b = c//2, row-half h = c%2 (2048 rows each).
Each core's Y is column-permuted host-side to [own-half | other-half] so the
identical SPMD program reads the X-half's stats from columns [0, 2048).
Row reductions are permutation-invariant, so the permutation is harmless.
"""

import sys

sys.path.insert(0, "/opt/trn_rl_repo")

import numpy as np
import ml_dtypes

import concourse.bass as bass
import concourse.tile as tile
from concourse import bacc
from concourse import mybir
from concourse.bass_utils import run_bass_kernel_spmd
from concourse.dve_ops import TENSOR_MASK_REDUCE

B = 4
C = 256
M = 4096  # 64*64 spatial positions
HALF = M // 2  # rows per core
NT = HALF // 128  # 16 i-tiles per core
N_CORES = 8
Q = 1024  # quarter width

F32 = mybir.dt.float32
BF16 = mybir.dt.bfloat16
AF = mybir.ActivationFunctionType
ALU = mybir.AluOpType


def build_nc() -> bass.Bass:
    nc = bacc.Bacc()

    x_d = nc.declare_dram_parameter("x", [C, HALF], BF16, isOutput=False)
    y_d = nc.declare_dram_parameter("y", [C, M], BF16, isOutput=False)
    v_d = nc.declare_dram_parameter("v", [128, NT], F32, isOutput=True)

    with tile.TileContext(nc) as tc:
        with (
            tc.tile_pool(name="io", bufs=1) as io,
            tc.tile_pool(name="consts", bufs=1) as consts,
            tc.tile_pool(name="stats", bufs=1) as stats,
            tc.tile_pool(name="psum_ka", bufs=1, space="PSUM") as psum_ka,
        ):
            y_q = [
                io.tile([128, 2, Q], BF16, name=f"y_q{i}") for i in range(4)
            ]
            x_sb = io.tile([128, 2, HALF], BF16)

            ones_col_bf = consts.tile([128, 1], BF16)
            nc.vector.memset(ones_col_bf, 1.0)
            bc_inv256 = consts.tile([128, 128], BF16)  # rank-reduce+bcast mu
            nc.vector.memset(bc_inv256, 1.0 / 256.0)
            bc_ones = consts.tile([128, 128], BF16)  # rank-reduce+bcast qy
            nc.vector.memset(bc_ones, 1.0)
            c1001_col = consts.tile([128, 1], F32)
            nc.vector.memset(c1001_col, 1.001)
            me_col = consts.tile([128, 1], F32)  # TMR mask_end: cover all
            nc.vector.memset(me_col, 1e9)
            ones_512 = consts.tile([128, 512], BF16)
            nc.vector.memset(ones_512, 1.0)

            sy16 = stats.tile([128, NT], F32)  # sum_c Y over own-half cols
            sx16 = stats.tile([128, NT], F32)  # sum_c X
            sxx16 = stats.tile([128, NT], F32)  # sum_c X^2
            nx2 = stats.tile([128, NT], F32)
            inv_nx = stats.tile([128, NT], F32)
            neg_inv_nx = stats.tile([128, NT], F32)
            ten_inv_nx = stats.tile([128, NT], F32)
            r16 = stats.tile([128, NT], F32)
            sumwA = stats.tile([128, NT], F32)
            maxw16 = stats.tile([128, NT], F32)
            rs16 = stats.tile([128, NT], F32)
            v16 = stats.tile([128, NT], F32)
            t16 = stats.tile([128, NT], F32)

            y_v = y_d.rearrange("(k p) m -> p k m", p=128)
            x_v = x_d.rearrange("(k p) m -> p k m", p=128)

            # keep-alive PSUM bank: dependency-free filler matmuls keep the
            # PE p-state from down-clocking between real bursts
            ka_ps = psum_ka.tile([128, 512], F32)

            def ka(n):
                for _ in range(n):
                    nc.tensor.matmul(
                        ka_ps[:, :], lhsT=bc_ones[:, :], rhs=ones_512[:, :],
                        start=True, stop=True,
                    )

            def tmr(out_ap, in_ap, accum_in, accum_out):
                nc.vector._custom_dve(
                    TENSOR_MASK_REDUCE,
                    out=out_ap, in0=in_ap, in1=me_col[:, :],
                    s0=0.0, s1=accum_in, imm2=1.0, accum_out=accum_out,
                )

            with (
                tc.tile_pool(name="pre", bufs=3, space="PSUM") as pre,
                tc.tile_pool(name="pstat", bufs=1, space="PSUM") as pstat,
                tc.tile_pool(name="scr", bufs=3) as scr,
            ):
                # ---- input DMAs: quarters land in arrival order; spread
                # descriptor-gen across HWDGE queues (transfers serialize on
                # the DMA engines anyway)
                for q in range(4):
                    eng = nc.sync if q % 2 == 0 else nc.scalar
                    eng.dma_start(
                        out=y_q[q][:, :, :], in_=y_v[:, :, q * Q : (q + 1) * Q]
                    )
                nc.gpsimd.dma_start(out=x_sb[:, :, 0:Q], in_=x_v[:, :, 0:Q])
                nc.gpsimd.dma_start(out=x_sb[:, :, Q:HALF], in_=x_v[:, :, Q:HALF])

                def stat16(dst, src_sb, tiles, ones):
                    """dst[p, t] = sum_c src[c, (t-tiles[0])*128 + p]."""
                    ps = pstat.tile([128, len(tiles)], F32, tag="pstat")
                    for i, t in enumerate(tiles):
                        for k in range(2):
                            nc.tensor.matmul(
                                ps[:, i : i + 1],
                                lhsT=src_sb[:, k, i * 128 : (i + 1) * 128],
                                rhs=ones[:, :],
                                start=(k == 0),
                                stop=(k == 1),
                            )
                    nc.scalar.activation(
                        dst[:, tiles[0] : tiles[0] + len(tiles)], ps[:, :],
                        AF.Copy,
                    )

                def center_mm(q):
                    # mu[p, j] = sum_c y[c, j] / 256 for every partition p
                    ps = pre.tile([128, Q], F32, tag="pre")
                    for j in range(2):
                        for k in range(2):
                            nc.tensor.matmul(
                                ps[:, j * 512 : (j + 1) * 512],
                                lhsT=bc_inv256[:, :],
                                rhs=y_q[q][:, k, j * 512 : (j + 1) * 512],
                                start=(k == 0),
                                stop=(k == 1),
                            )
                    return ps

                def center_sub_dve(q, ps):
                    for k in range(2):
                        nc.vector.tensor_sub(
                            y_q[q][:, k, :], y_q[q][:, k, :], ps[:, :]
                        )

                def center_sub_pool(q, ps):
                    # GPSIMD can't read PSUM: ScalarE stages mu to SBUF bf16
                    mu_bf = scr.tile([128, Q], BF16, tag="mubf")
                    nc.scalar.activation(mu_bf[:, :], ps[:, :], AF.Copy)
                    for k in range(2):
                        nc.gpsimd.tensor_sub(
                            y_q[q][:, k, :], y_q[q][:, k, :], mu_bf[:, :]
                        )

                def sq_quarter(q, on_dve):
                    sq = scr.tile([128, 2, Q], BF16, tag="sq")
                    src = y_q[q][:, :, :]
                    if on_dve:
                        nc.vector.tensor_mul(sq[:, :, :], src, src)
                    else:
                        nc.scalar.activation(sq[:, :, :], src, AF.Square)
                    return sq

                def ny_quarter(q, sq, scale_pool=False):
                    # qy[p, j] = sum_c Yc[c, j]^2 broadcast via ones lhsT;
                    # inv_ny = 1/sqrt(qy) in one ScalarE op from PSUM, then
                    # y_q *= inv_ny (bf16 2x)
                    ps = pre.tile([128, Q], F32, tag="pre")
                    for j in range(2):
                        for k in range(2):
                            nc.tensor.matmul(
                                ps[:, j * 512 : (j + 1) * 512],
                                lhsT=bc_ones[:, :],
                                rhs=sq[:, k, j * 512 : (j + 1) * 512],
                                start=(k == 0),
                                stop=(k == 1),
                            )
                    ny_bf = scr.tile([128, Q], BF16, tag="nybf")
                    nc.scalar.activation(
                        ny_bf[:, :], ps[:, :], AF.Abs_reciprocal_sqrt
                    )
                    eng = nc.gpsimd if scale_pool else nc.vector
                    for k in range(2):
                        eng.tensor_mul(
                            y_q[q][:, k, :], y_q[q][:, k, :], ny_bf[:, :]
                        )

                # ---- phase schedule -----------------------------------
                # dummy activations: pull the Exp/Abs_reciprocal_sqrt act
                # tables in during the DMA wait instead of mid-pipeline
                dum = scr.tile([128, 1], F32, tag="dum")
                nc.scalar.activation(
                    dum[:, :], c1001_col[:, :], AF.Abs_reciprocal_sqrt
                )
                # own-half raw-Y column sums (must precede centering)
                cps = [None] * 4
                cps[0] = center_mm(0)
                stat16(sy16, y_q[0], list(range(0, 8)), ones_col_bf)
                center_sub_dve(0, cps[0])
                ka(10)  # ramp the PE clock behind the first mu burst
                cps[1] = center_mm(1)
                stat16(sy16, y_q[1], list(range(8, 16)), ones_col_bf)
                center_sub_dve(1, cps[1])
                cps[2] = center_mm(2)
                center_sub_pool(2, cps[2])
                cps[3] = center_mm(3)
                center_sub_pool(3, cps[3])
                sq0 = sq_quarter(0, on_dve=True)
                sq1 = sq_quarter(1, on_dve=False)
                sq2 = sq_quarter(2, on_dve=False)
                sq3 = sq_quarter(3, on_dve=True)
                ny_quarter(0, sq0)
                ny_quarter(1, sq1)
                ny_quarter(2, sq2)
                ny_quarter(3, sq3)
                # X stats: x lands last; deprioritize so early main-loop
                # work can slot ahead in the engine queues
                with tc.tile_wait_until(0.0105):
                    sqx = scr.tile([128, 2, HALF], BF16, tag="sqx")
                    nc.vector.tensor_mul(sqx[:, :, :], x_sb[:, :, :], x_sb[:, :, :])
                    stat16(sx16, x_sb, list(range(NT)), ones_col_bf)
                    stat16(sxx16, sqx, list(range(NT)), ones_col_bf)
                # nx2 = sxx - (sy/256)*(2*sx - sy)  (tiny DVE ops)
                nc.vector.tensor_scalar(
                    out=t16[:, :], in0=sx16[:, :], scalar1=2.0, scalar2=None,
                    op0=ALU.mult,
                )
                nc.vector.tensor_sub(t16[:, :], t16[:, :], sy16[:, :])
                nc.vector.tensor_mul(t16[:, :], t16[:, :], sy16[:, :])
                nc.vector.tensor_scalar(
                    out=t16[:, :], in0=t16[:, :], scalar1=1.0 / 256.0,
                    scalar2=None, op0=ALU.mult,
                )
                nc.vector.tensor_sub(nx2[:, :], sxx16[:, :], t16[:, :])
                nc.scalar.activation(
                    inv_nx[:, :], nx2[:, :], AF.Abs_reciprocal_sqrt
                )  # 1/|Xc|
                nc.vector.tensor_scalar(
                    out=neg_inv_nx[:, :], in0=inv_nx[:, :], scalar1=-1.0,
                    scalar2=None, op0=ALU.mult,
                )
                nc.vector.tensor_scalar(
                    out=ten_inv_nx[:, :], in0=inv_nx[:, :], scalar1=10.0,
                    scalar2=None, op0=ALU.mult,
                )

            # ---- main loop -------------------------------------------------
            with (
                tc.tile_pool(name="psum_g", bufs=3, space="PSUM") as psum_g,
                tc.tile_pool(name="dpool", bufs=5) as dpool,
                tc.tile_pool(name="wpool", bufs=1) as wpool,
                tc.tile_pool(name="mpool", bufs=4) as mpool,
                tc.tile_pool(name="mains", bufs=10) as mains,
            ):

                def quarter_mm(t, g):
                    ps = psum_g.tile([128, Q], F32, tag="g")
                    for k in range(2):
                        for j in range(2):
                            nc.tensor.matmul(
                                ps[:, j * 512 : (j + 1) * 512],
                                lhsT=x_sb[:, k, t * 128 : (t + 1) * 128],
                                rhs=y_q[g][:, k, j * 512 : (j + 1) * 512],
                                start=(k == 0),
                                stop=(k == 1),
                            )
                    return ps

                def q3_tree(t, d_sb):
                    # quarter 3 (Act-evicted, no fused max): pairwise bf16
                    # 2x tree + short reduce
                    m1 = mpool.tile([128, 512], BF16, tag="m1")
                    m2 = mpool.tile([128, 128], BF16, tag="m2")
                    mx3 = mains.tile([128, 1], F32, tag="mx3")
                    nc.vector.tensor_tensor(
                        out=m1[:, :], in0=d_sb[:, 3072:3584],
                        in1=d_sb[:, 3584:4096], op=ALU.max,
                    )
                    nc.vector.tensor_tensor(
                        out=m2[:, :], in0=m1[:, 0:128], in1=m1[:, 128:256],
                        op=ALU.max,
                    )
                    nc.vector.tensor_tensor(
                        out=m2[:, :], in0=m2[:, :], in1=m1[:, 256:384],
                        op=ALU.max,
                    )
                    nc.vector.tensor_tensor(
                        out=m2[:, :], in0=m2[:, :], in1=m1[:, 384:512],
                        op=ALU.max,
                    )
                    nc.vector.reduce_max(
                        mx3, m2[:, :], axis=mybir.AxisListType.X
                    )
                    return mx3

                def exp_tile(t, d_sb, scale_i, bias_i):
                    w_sb = wpool.tile([128, M], BF16, tag="w")
                    nc.scalar.activation(
                        out=w_sb[:, :],
                        in_=d_sb[:, :],
                        func=AF.Exp,
                        bias=bias_i,
                        scale=scale_i,
                        accum_out=sumwA[:, t : t + 1],
                    )

                prev = None
                ps_pend = [quarter_mm(0, g) for g in range(3)]
                for t in range(NT):
                    d_sb = dpool.tile([128, M], BF16, tag="d")
                    ps0, ps1, ps2 = ps_pend
                    ka(2)
                    ps3 = quarter_mm(t, 3)
                    ka(2)
                    m0 = mains.tile([128, 1], F32, tag="m0")
                    m1c = mains.tile([128, 1], F32, tag="m1c")
                    m2c = mains.tile([128, 1], F32, tag="m2c")
                    tmr(d_sb[:, 0:Q], ps0[:, :], -1e30, m0)
                    tmr(d_sb[:, Q : 2 * Q], ps1[:, :], m0, m1c)
                    if prev is not None:
                        pt, pd, psc, pbi = prev
                        exp_tile(pt, pd, psc, pbi)
                    last = t == NT - 1 or t == 0
                    if not last:
                        # ScalarE evicts quarter 3 (pure copy; exp runs first
                        # in the Act queue)
                        nc.scalar.activation(
                            d_sb[:, 3 * Q : M], ps3[:, :], AF.Copy
                        )
                    tmr(d_sb[:, 2 * Q : 3 * Q], ps2[:, :], m1c, m2c)
                    if t + 1 < NT:
                        ps_pend = [quarter_mm(t + 1, g) for g in range(3)]
                    smax = mains.tile([128, 1], F32, tag="smax")
                    if last:
                        # final tile: fused TMR keeps the tail off ScalarE
                        tmr(d_sb[:, 3 * Q : M], ps3[:, :], m2c, smax)
                    else:
                        mx3 = q3_tree(t, d_sb)
                        nc.vector.tensor_tensor(
                            out=smax, in0=m2c, in1=mx3, op=ALU.max
                        )
                    # u = 1.001 - smax/|Xc_i| ; r = 1/u
                    u = mains.tile([128, 1], F32, tag="u")
                    nc.vector.scalar_tensor_tensor(
                        out=u, in0=smax, scalar=neg_inv_nx[:, t : t + 1],
                        in1=c1001_col[:, :], op0=ALU.mult, op1=ALU.add,
                    )
                    nc.vector.reciprocal(r16[:, t : t + 1], u)
                    # scale_i = 10*r/|Xc_i| on GPSIMD; bias_i = 10 - 10r
                    scale_i = mains.tile([128, 1], F32, tag="scale")
                    bias_i = mains.tile([128, 1], F32, tag="bias")
                    nc.gpsimd.tensor_mul(
                        scale_i, r16[:, t : t + 1], ten_inv_nx[:, t : t + 1]
                    )
                    nc.vector.tensor_scalar(
                        out=bias_i, in0=r16[:, t : t + 1], scalar1=-10.0,
                        scalar2=10.0, op0=ALU.mult, op1=ALU.add,
                    )
                    prev = (t, d_sb, scale_i, bias_i)

                pt, pd, psc, pbi = prev
                exp_tile(pt, pd, psc, pbi)

                # ---- epilogue: v = exp(0.01*r) / sumw ---------------------
                nc.scalar.activation(maxw16[:, :], r16[:, :], AF.Exp, scale=0.01)
                nc.vector.reciprocal(rs16[:, :], sumwA[:, :])
                nc.vector.tensor_mul(v16[:, :], maxw16[:, :], rs16[:, :])
                nc.sync.dma_start(out=v_d[:, :], in_=v16[:, :])

    nc.compile()
    return nc


_NC = None


def _get_nc():
    global _NC
    if _NC is None:
        _NC = build_nc()
    return _NC


def make_in_maps(X, Y):
    """Per-core bf16 inputs. Y columns permuted to [own-half | other-half]."""
    Xb = X.astype(ml_dtypes.bfloat16)
    Yb = Y.astype(ml_dtypes.bfloat16)
    in_maps = []
    for c in range(N_CORES):
        b, h = c // 2, c % 2
        xs = np.ascontiguousarray(Xb[b][:, h * HALF : (h + 1) * HALF])
        ys = np.ascontiguousarray(
            np.concatenate(
                [
                    Yb[b][:, h * HALF : (h + 1) * HALF],
                    Yb[b][:, (1 - h) * HALF : (2 - h) * HALF],
                ],
                axis=1,
            )
        )
        in_maps.append({"x": xs, "y": ys})
    return in_maps


def finish_host(results):
    """results: list of 8 per-core dicts with 'v' [128, NT]."""
    cx = np.zeros(B, dtype=np.float64)
    for c in range(N_CORES):
        cx[c // 2] += np.asarray(results[c]["v"]).astype(np.float64).sum()
    cx /= M
    return np.float32(np.mean(-np.log(cx)))


def run(X_features, Y_features, trace=False, tmpdir=None):
    X = np.asarray(X_features, dtype=np.float32).reshape(B, C, M)
    Y = np.asarray(Y_features, dtype=np.float32).reshape(B, C, M)
    nc = _get_nc()
    res = run_bass_kernel_spmd(
        nc, make_in_maps(X, Y), list(range(N_CORES)), trace=trace, tmpdir=tmpdir
    )
    return finish_host(res.results), res


def kernel(X_features, Y_features):
    loss, _ = run(X_features, Y_features)
    return loss
